# revision 1
# baseline (speedup 1.0000x reference)
"""Trainium2 Bass kernel for nn_Detail_loss (histogram_binning).

Data-parallel over B=32 samples -> 8 cores x 4 samples. Each core:
  1. 5x5 binary dilation of labels -> mask (PE banded matmuls vertical,
     row-cumsum difference trick horizontal).
  2. Masked 256-bin histogram of images*mask*255 (torch.histc semantics)
     via 16x16 hi/lo one-hot factorization: hist2d[h,l] = sum_p
     m_p*[hi_p==h]*[lo_p==l], computed as PE outer-product matmuls over
     bf16 one-hot planes.
  3. Two-threshold Otsu argmax over the 254x254 grid (first max,
     row-major). bv2/bv0 terms are t2-/t1-separable rows/cols; only the
     mean1 term is a true 2D grid. Divisions via the exact HW reciprocal
     (0 ulp) / reciprocal_approx_accurate (2 ulp; top-2 bv gaps are
     ~0.004+ vs ~1e-4 noise). Exact thresholds fl(k/255) via a
     Markstein-corrected table. floor() via the 2^23 round-trip.
  4. ci = max([im>=t2], 0.5*[im>=t1]) (== nested where); per-sample
     sq = sum((ci - preds*mask)^2), sm = sum(mask).
Host: loss = mean over valid samples of sq/sm (np.float32 math).
"""

import os

import numpy as np

import concourse.bass as bass
import concourse.mybir as mybir
from concourse import bacc, bass_isa, tile
from concourse.bass_utils import run_bass_kernel_spmd

F32 = mybir.dt.float32
BF16 = mybir.dt.bfloat16
I32 = mybir.dt.int32
OP = mybir.AluOpType
ACT = mybir.ActivationFunctionType
AX = mybir.AxisListType

STAGE = int(os.environ.get("KSTAGE", "9"))
B_PER_CORE = 4
H = 512
W = 512
NSLAB = 4
NBINS = 256
NT = 254
BIG = 4194304.0      # 2^22: BIG+flat stays integer-exact in f32
MAGIC = 8388608.0    # 2^23 round-to-integer magic
EPS = 1e-8

C_BIN = float(np.float32(NBINS / 255.0))     # fl(256/255), exact in f64
S1 = 255.0
R254 = float(np.float32(1.0) / np.float32(254.0))

# engine per one-hot plane (32 total: 16 A then 16 B)
PLANE_ENG = (["dve"] * 5 + ["pool"] * 7 + ["act"] * 4 +
             ["dve"] * 5 + ["pool"] * 6 + ["act"] * 5)
assert len(PLANE_ENG) == 32


def build_nc():
    nc = bacc.Bacc("TRN2", target_bir_lowering=False)

    lab_d = nc.dram_tensor("labels", [B_PER_CORE * H, W], F32, kind="ExternalInput")
    img_d = nc.dram_tensor("images", [B_PER_CORE * H, W], F32, kind="ExternalInput")
    prd_d = nc.dram_tensor("preds", [B_PER_CORE * H, W], F32, kind="ExternalInput")
    # out[0, 4b+s] = partial sq (sample b, slab s); out[0, 16+4b+s] = partial sm
    out_d = nc.dram_tensor("stats", [1, 32], F32, kind="ExternalOutput")
    dbg_d = nc.dram_tensor("dbg", [1, 16], F32, kind="ExternalOutput")

    with tile.TileContext(nc) as tc:
        _emit(nc, tc, lab_d, img_d, prd_d, out_d, dbg_d)
    nc.compile()
    return nc


def _sample_view(dram, b):
    return dram[512 * b:512 * (b + 1), :].rearrange("(s p) c -> p s c", p=128)


def _floor(nc, eng, out_ap, in_ap, tmp_ap):
    """out = floor(in) for 0 <= in < 2^22, exact. tmp is f32 scratch."""
    eng.tensor_scalar(out_ap, in_ap, MAGIC, MAGIC, OP.add, OP.subtract)
    eng.tensor_tensor(tmp_ap, out_ap, in_ap, OP.is_gt)
    eng.tensor_tensor(out_ap, out_ap, tmp_ap, OP.subtract)


def _emit(nc, tc, lab_d, img_d, prd_d, out_d, dbg_d):
    import contextlib
    ctx = contextlib.ExitStack()
    with ctx:
        const = ctx.enter_context(tc.tile_pool(name="const", bufs=1))
        lab_pool = ctx.enter_context(tc.tile_pool(name="lab", bufs=2))
        labb_pool = ctx.enter_context(tc.tile_pool(name="labb", bufs=2))
        img_pool = ctx.enter_context(tc.tile_pool(name="img", bufs=2))
        prd_pool = ctx.enter_context(tc.tile_pool(name="prd", bufs=2))
        m_pool = ctx.enter_context(tc.tile_pool(name="mask", bufs=2))
        scr_pool = ctx.enter_context(tc.tile_pool(name="scr", bufs=2))
        plane_pool = ctx.enter_context(tc.tile_pool(name="planes", bufs=2))
        otsu_pool = ctx.enter_context(tc.tile_pool(name="otsu", bufs=1))
        stat_pool = ctx.enter_context(tc.tile_pool(name="stat", bufs=1))
        vpsum = ctx.enter_context(
            tc.tile_pool(name="vpsum", bufs=3, space=bass.MemorySpace.PSUM))
        hpsum = ctx.enter_context(
            tc.tile_pool(name="hpsum", bufs=2, space=bass.MemorySpace.PSUM))

        # ---------------- constants ----------------
        io_fp = const.tile([128, 128], I32, tag="io_fp")   # f - p
        nc.gpsimd.iota(io_fp[:], pattern=[[1, 128]], base=0, channel_multiplier=-1)
        io_pf = const.tile([128, 128], I32, tag="io_pf")   # p - f
        nc.gpsimd.iota(io_pf[:], pattern=[[-1, 128]], base=0, channel_multiplier=1)

        bv_band = const.tile([128, 128], BF16, tag="bv_band")
        btmp = const.tile([128, 128], F32, tag="btmp")
        nc.vector.tensor_scalar(btmp[:], io_fp[:], -2, None, OP.is_ge)
        nc.vector.scalar_tensor_tensor(bv_band[:], io_fp[:], 2, btmp[:], OP.is_le, OP.mult)
        up_band = const.tile([128, 128], BF16, tag="up_band")
        nc.vector.tensor_scalar(up_band[:], io_pf[:], 126, None, OP.is_ge)
        dn_band = const.tile([128, 128], BF16, tag="dn_band")
        nc.vector.tensor_scalar(dn_band[:], io_fp[:], 126, None, OP.is_ge)

        io256 = const.tile([1, 256], F32, tag="io256")     # 0..255
        nc.gpsimd.iota(io256[:], pattern=[[1, 256]], base=0, channel_multiplier=0,
                       allow_small_or_imprecise_dtypes=True)
        iot = const.tile([1, NT], F32, tag="iot")          # 0..253
        nc.gpsimd.iota(iot[:], pattern=[[1, NT]], base=0, channel_multiplier=0,
                       allow_small_or_imprecise_dtypes=True)
        iobig = const.tile([127, NT], F32, tag="iobig")    # t2 + BIG
        nc.gpsimd.iota(iobig[:], pattern=[[1, NT]], base=0, channel_multiplier=0,
                       allow_small_or_imprecise_dtypes=True)
        nc.vector.tensor_scalar(iobig[:], iobig[:], BIG, None, OP.add)
        fbase = const.tile([127, 2], F32, tag="fbase")     # 254*p + 127*254*h
        nc.gpsimd.iota(fbase[:], pattern=[[127 * 254, 2]], base=0,
                       channel_multiplier=254, allow_small_or_imprecise_dtypes=True)

        # exact threshold table T[t] = fl((t+1)/255), t = 0..253 (Markstein)
        c255 = const.tile([1, 1], F32, tag="c255")
        nc.vector.memset(c255[:], 255.0)
        r255 = const.tile([1, 1], F32, tag="r255")
        nc.vector.reciprocal(r255[:], c255[:])
        iok = const.tile([1, NT], F32, tag="iok")          # 1..254
        nc.gpsimd.iota(iok[:], pattern=[[1, NT]], base=1, channel_multiplier=0,
                       allow_small_or_imprecise_dtypes=True)
        Ttab = const.tile([1, NT], F32, tag="Ttab")
        tA = const.tile([1, NT], F32, tag="tA")
        tS = const.tile([1, NT], F32, tag="tS")
        tD = const.tile([1, NT], F32, tag="tD")
        nc.vector.tensor_scalar(Ttab[:], iok[:], r255[:], None, OP.mult)   # q0
        nc.vector.tensor_scalar(tA[:], Ttab[:], 256.0, None, OP.mult)
        nc.vector.tensor_tensor(tS[:], tA[:], Ttab[:], OP.subtract)
        nc.vector.tensor_tensor(tD[:], tA[:], tS[:], OP.subtract)
        nc.vector.tensor_tensor(tD[:], tD[:], Ttab[:], OP.subtract)        # err
        nc.vector.tensor_tensor(tS[:], iok[:], tS[:], OP.subtract)         # k-s
        nc.vector.tensor_tensor(tS[:], tS[:], tD[:], OP.subtract)          # e
        nc.vector.tensor_scalar(tS[:], tS[:], r255[:], None, OP.mult)
        nc.vector.tensor_tensor(Ttab[:], Ttab[:], tS[:], OP.add)

        bias_tiles = {}

        def bias_ap(val, p=128):
            v = float(np.float32(val))
            if v not in bias_tiles:
                t = const.tile([128, 1], F32, tag=f"bias{len(bias_tiles)}")
                nc.vector.memset(t[:], v)
                bias_tiles[v] = t
            return bias_tiles[v][0:p, :]

        sq_cols = stat_pool.tile([128, 16], F32, tag="sq_cols")
        sm_cols = stat_pool.tile([128, 16], F32, tag="sm_cols")
        dbg_row = stat_pool.tile([1, 16], F32, tag="dbg_row")
        nc.vector.memset(sq_cols[:], 0.0)
        nc.vector.memset(sm_cols[:], 0.0)
        nc.vector.memset(dbg_row[:], 0.0)

        for b in range(B_PER_CORE):
            # ---------------- load ----------------
            lab = lab_pool.tile([128, 4 * W], F32, tag="lab")
            nc.sync.dma_start(out=lab[:].rearrange("p (s c) -> p s c", s=4),
                              in_=_sample_view(lab_d, b))
            img = img_pool.tile([128, 4 * W], F32, tag="img")
            nc.sync.dma_start(out=img[:].rearrange("p (s c) -> p s c", s=4),
                              in_=_sample_view(img_d, b))

            labb = labb_pool.tile([128, 4 * W], BF16, tag="labb")
            for s in range(NSLAB):
                nc.scalar.activation(labb[:, 512 * s:512 * (s + 1)],
                                     lab[:, 512 * s:512 * (s + 1)], ACT.Copy)

            M = m_pool.tile([128, 4 * W], F32, tag="M")
            hist = hpsum.tile([16, 16], F32, tag="hist")

            for s in range(NSLAB):
                sl = slice(512 * s, 512 * (s + 1))
                # ------- vertical 5-conv (PE banded) -------
                yv = vpsum.tile([128, W], F32, tag="yv")
                mms = [(bv_band, s)]
                if s > 0:
                    mms.append((up_band, s - 1))
                if s < NSLAB - 1:
                    mms.append((dn_band, s + 1))
                for i, (band, src) in enumerate(mms):
                    nc.tensor.matmul(
                        yv[:], band[:], labb[:, 512 * src:512 * (src + 1)],
                        start=(i == 0), stop=(i == len(mms) - 1))

                # ------- horizontal via row-cumsum difference -------
                cp = scr_pool.tile([128, 520], F32, tag="cp")
                nc.vector.memset(cp[:, 0:3], 0.0)
                nc.vector.tensor_tensor_scan(
                    cp[:, 3:515], yv[:], lab[:, sl], 0.0, OP.add, OP.bypass)
                nc.vector.tensor_copy(out=cp[:, 515:516], in_=cp[:, 514:515])
                nc.vector.tensor_copy(out=cp[:, 516:517], in_=cp[:, 514:515])
                nc.vector.scalar_tensor_tensor(
                    M[:, sl], cp[:, 5:517], 0.0, cp[:, 0:512],
                    OP.add, OP.is_gt,
                    accum_out=sm_cols[:, 4 * b + s:4 * b + s + 1])
                if STAGE < 2:
                    continue
                # ------- bin index (exact reference arithmetic) -------
                nc.vector.tensor_tensor(img[:, sl], img[:, sl], M[:, sl], OP.mult)
                v = scr_pool.tile([128, W], F32, tag="t4")
                nc.scalar.activation(v[:], img[:, sl], ACT.Copy, scale=S1)
                w = scr_pool.tile([128, W], F32, tag="t0")
                nc.scalar.activation(w[:], v[:], ACT.Copy, scale=C_BIN)
                idx = scr_pool.tile([128, W], F32, tag="t1")
                tmpf = scr_pool.tile([128, W], F32, tag="t3")
                _floor(nc, nc.vector, idx[:], w[:], tmpf[:])
                nc.vector.tensor_scalar(idx[:], idx[:], 255.0, None, OP.min)
                # h+16 via bias trick: RN((idx-7.5)/16 + 16) == floor(idx/16)+16
                q = scr_pool.tile([128, W], F32, tag="t2")
                nc.scalar.activation(q[:], idx[:], ACT.Copy, scale=0.0625, bias=15.53125)
                h16 = scr_pool.tile([128, W], BF16, tag="hi")
                nc.vector.tensor_scalar(h16[:], q[:], MAGIC, MAGIC, OP.add, OP.subtract)
                # hi' = h16 - 16*M: masked -> h (0..15), unmasked -> h+16 (out of range)
                hip = scr_pool.tile([128, W], BF16, tag="hip")
                nc.vector.scalar_tensor_tensor(hip[:], M[:, sl], -16.0, h16[:], OP.mult, OP.add)
                # lo' = idx - 16*h16 = lo - 256
                lo = scr_pool.tile([128, W], BF16, tag="lo")
                nc.vector.scalar_tensor_tensor(lo[:], h16[:], -16.0, idx[:], OP.mult, OP.add)

                # ------- one-hot planes (bf16), split across DVE/Pool/ACT -------
                A = plane_pool.tile([128, 16 * W], BF16, tag="A")
                Bp = plane_pool.tile([128, 16 * W], BF16, tag="B")
                bump = scr_pool.tile([128, W], F32, tag="bump")
                for j in range(16):
                    pl = slice(512 * j, 512 * (j + 1))
                    eng = PLANE_ENG[j]
                    if eng == "dve":
                        nc.vector.tensor_scalar(A[:, pl], hip[:], float(j), None, OP.is_equal)
                    elif eng == "pool":
                        nc.gpsimd.tensor_scalar(A[:, pl], hip[:], float(j), None, OP.is_equal)
                    else:
                        nc.scalar.activation(bump[:], hip[:], ACT.Square, bias=bias_ap(-j))
                        nc.scalar.activation(A[:, pl], bump[:], ACT.Relu, scale=-1.0, bias=1.0)
                for j in range(16):
                    pl = slice(512 * j, 512 * (j + 1))
                    eng = PLANE_ENG[16 + j]
                    jv = float(j - 256)
                    if eng == "dve":
                        nc.vector.tensor_scalar(Bp[:, pl], lo[:], jv, None, OP.is_equal)
                    elif eng == "pool":
                        nc.gpsimd.tensor_scalar(Bp[:, pl], lo[:], jv, None, OP.is_equal)
                    else:
                        nc.scalar.activation(bump[:], lo[:], ACT.Square, bias=bias_ap(-jv))
                        nc.scalar.activation(Bp[:, pl], bump[:], ACT.Relu, scale=-1.0, bias=1.0)

                # ------- PE outer-product accumulation -------
                Ac = A[:].rearrange("p (j c) -> p c j", j=16)
                Bc = Bp[:].rearrange("p (j c) -> p c j", j=16)
                for c in range(W):
                    nc.tensor.matmul(
                        hist[:], Ac[:, c, :], Bc[:, c, :],
                        start=(s == 0 and c == 0),
                        stop=(s == NSLAB - 1 and c == W - 1))

            # ---------------- Otsu ----------------
            if STAGE < 3:
                continue
            hist_s = otsu_pool.tile([16, 16], F32, tag="hist_s")
            nc.vector.tensor_copy(out=hist_s[:], in_=hist[:])
            hrow = otsu_pool.tile([1, 256], F32, tag="hrow")
            nc.sync.dma_start(out=hrow[:], in_=hist_s[:])
            ntot = otsu_pool.tile([1, 1], F32, tag="ntot")
            nc.vector.tensor_reduce(ntot[:], hrow[:], AX.X, OP.add)
            rn = otsu_pool.tile([1, 1], F32, tag="rn")
            nc.vector.reciprocal(rn[:], ntot[:])
            hn = otsu_pool.tile([1, 256], F32, tag="hn")
            nc.vector.tensor_scalar(hn[:], hrow[:], rn[:], None, OP.mult)
            ch = otsu_pool.tile([1, 256], F32, tag="ch")
            nc.vector.tensor_tensor_scan(ch[:], hn[:], hn[:], 0.0, OP.add, OP.bypass)
            hj = otsu_pool.tile([1, 256], F32, tag="hj")
            nc.vector.tensor_tensor(hj[:], hn[:], io256[:], OP.mult)
            cm = otsu_pool.tile([1, 256], F32, tag="cm")
            nc.vector.tensor_tensor_scan(cm[:], hj[:], hj[:], 0.0, OP.add, OP.bypass)

            if STAGE < 4:
                continue
            # t2-separable row terms: w2, bv2, vw2  (partition 0)
            w2r = otsu_pool.tile([1, NT], F32, tag="w2r")
            nc.vector.tensor_scalar(w2r[:], ch[0:1, 0:NT], -1.0, 1.0, OP.mult, OP.add)
            w2pr = otsu_pool.tile([1, NT], F32, tag="w2pr")
            nc.vector.tensor_scalar(w2pr[:], w2r[:], EPS, None, OP.add)
            r2r = otsu_pool.tile([1, NT], F32, tag="r2r")
            rscr = otsu_pool.tile([1, NT], F32, tag="rscr")
            nc.vector.reciprocal_approx_accurate(r2r[:], w2pr[:], rscr[:])
            tm_ap = cm[0:1, 255:256]
            m2r = otsu_pool.tile([1, NT], F32, tag="m2r")
            nc.vector.tensor_scalar(m2r[:], cm[0:1, 0:NT], -1.0, tm_ap, OP.mult, OP.add)
            nc.vector.tensor_tensor(m2r[:], m2r[:], r2r[:], OP.mult)       # mean2
            nc.vector.tensor_scalar(m2r[:], m2r[:], tm_ap, None, OP.subtract)
            nc.vector.tensor_tensor(m2r[:], m2r[:], m2r[:], OP.mult)
            bv2r = otsu_pool.tile([1, NT], F32, tag="bv2r")
            nc.vector.tensor_tensor(bv2r[:], m2r[:], w2r[:], OP.mult)
            vw2r = otsu_pool.tile([1, NT], F32, tag="vw2r")
            nc.vector.tensor_scalar(vw2r[:], w2r[:], 0.0, None, OP.is_gt)
            nc.vector.tensor_tensor(bv2r[:], bv2r[:], vw2r[:], OP.mult)

            bv2b = otsu_pool.tile([127, NT], F32, tag="bv2b")
            nc.gpsimd.partition_broadcast(bv2b[:], bv2r[:], channels=127)
            vw2b = otsu_pool.tile([127, NT], F32, tag="vw2b")
            nc.gpsimd.partition_broadcast(vw2b[:], vw2r[:], channels=127)
            tmcol = otsu_pool.tile([127, 1], F32, tag="tmcol")
            nc.gpsimd.partition_broadcast(tmcol[:], tm_ap, channels=127)
            ab = otsu_pool.tile([127, NT], F32, tag="ab")
            nc.gpsimd.partition_broadcast(ab[:], ch[0:1, 0:NT], channels=127)
            bb = otsu_pool.tile([127, NT], F32, tag="bb")
            nc.gpsimd.partition_broadcast(bb[:], cm[0:1, 0:NT], channels=127)

            acol = otsu_pool.tile([127, 2], F32, tag="acol")
            bcol = otsu_pool.tile([127, 2], F32, tag="bcol")
            for hh in range(2):
                rs = slice(127 * hh, 127 * (hh + 1))
                nc.sync.dma_start(out=acol[:, hh:hh + 1], in_=ch[0:1, rs])
                nc.sync.dma_start(out=bcol[:, hh:hh + 1], in_=cm[0:1, rs])

            colmax2 = otsu_pool.tile([127, 2], F32, tag="colmax2")
            t2min2 = otsu_pool.tile([127, 2], F32, tag="t2min2")
            for hh in range(2):
                a_c = acol[:, hh:hh + 1]
                b_c = bcol[:, hh:hh + 1]
                # t1-separable column terms: bv0, vw0
                w0p = otsu_pool.tile([127, 1], F32, tag="w0p")
                nc.vector.tensor_scalar(w0p[:], a_c, EPS, None, OP.add)
                r0c = otsu_pool.tile([127, 1], F32, tag="r0c")
                r0s = otsu_pool.tile([127, 1], F32, tag="r0s")
                nc.vector.reciprocal_approx_accurate(r0c[:], w0p[:], r0s[:])
                d0 = otsu_pool.tile([127, 1], F32, tag="d0")
                nc.vector.tensor_tensor(d0[:], b_c, r0c[:], OP.mult)       # mean0
                nc.vector.tensor_scalar(d0[:], d0[:], tmcol[:], None, OP.subtract)
                nc.vector.tensor_tensor(d0[:], d0[:], d0[:], OP.mult)
                nc.vector.tensor_scalar(d0[:], d0[:], a_c, None, OP.mult)  # bv0
                vw0 = otsu_pool.tile([127, 1], F32, tag="vw0")
                nc.vector.tensor_scalar(vw0[:], a_c, 0.0, None, OP.is_gt)

                # 2D mean1 term (elementwise adds/squares on ACT)
                w1 = otsu_pool.tile([127, NT], F32, tag="w1")
                nc.vector.tensor_scalar(w1[:], ab[:], a_c, None, OP.subtract)
                w1p = otsu_pool.tile([127, NT], F32, tag="w1p")
                nc.scalar.activation(w1p[:], w1[:], ACT.Copy, bias=float(np.float32(EPS)))
                rw1 = otsu_pool.tile([127, NT], F32, tag="rw1")
                rw1s = otsu_pool.tile([127, NT], F32, tag="rw1s")
                nc.vector.reciprocal_approx_accurate(rw1[:], w1p[:], rw1s[:])
                d1 = otsu_pool.tile([127, NT], F32, tag="d1")
                nc.vector.tensor_scalar(d1[:], bb[:], b_c, None, OP.subtract)
                nc.vector.tensor_tensor(d1[:], d1[:], rw1[:], OP.mult)     # mean1
                nc.vector.tensor_scalar(d1[:], d1[:], tmcol[:], None, OP.subtract)
                nc.vector.tensor_tensor(d1[:], d1[:], d1[:], OP.mult)
                bv = otsu_pool.tile([127, NT], F32, tag="bv")
                nc.vector.tensor_tensor(bv[:], d1[:], w1[:], OP.mult)      # bv1
                vw1 = otsu_pool.tile([127, NT], F32, tag="vw1")
                nc.vector.tensor_scalar(vw1[:], w1[:], 0.0, None, OP.is_gt)

                # bv = ((bv0 + bv1) + bv2) * vw0*vw1*vw2
                nc.vector.tensor_scalar(bv[:], bv[:], d0[:], None, OP.add)
                nc.vector.tensor_tensor(bv[:], bv[:], bv2b[:], OP.add)
                nc.vector.tensor_tensor(bv[:], bv[:], vw1[:], OP.mult)
                nc.vector.tensor_tensor(bv[:], bv[:], vw2b[:], OP.mult)
                nc.vector.tensor_scalar(bv[:], bv[:], vw0[:], None, OP.mult)

                cmx = colmax2[:, hh:hh + 1]
                nc.vector.tensor_reduce(cmx, bv[:], AX.X, OP.max)
                eq = otsu_pool.tile([127, NT], F32, tag="eq")
                nc.vector.tensor_scalar(eq[:], bv[:], cmx, None, OP.is_equal)
                nc.vector.scalar_tensor_tensor(
                    eq[:], eq[:], -BIG, iobig[:], OP.mult, OP.add)
                nc.vector.tensor_reduce(t2min2[:, hh:hh + 1], eq[:], AX.X, OP.min)

            # global first-max across [127, 2]
            gmax = otsu_pool.tile([127, 1], F32, tag="gmax")
            nc.vector.tensor_reduce(gmax[:], colmax2[:], AX.X, OP.max)
            nc.gpsimd.partition_all_reduce(gmax[:], gmax[:], channels=127,
                                           reduce_op=bass_isa.ReduceOp.max)
            flat = otsu_pool.tile([127, 2], F32, tag="flat")
            nc.vector.tensor_tensor(flat[:], t2min2[:], fbase[:], OP.add)
            nfb = otsu_pool.tile([127, 2], F32, tag="nfb")
            nc.vector.tensor_scalar(nfb[:], flat[:], -1.0, -BIG, OP.mult, OP.add)
            eqg = otsu_pool.tile([127, 2], F32, tag="eqg")
            nc.vector.tensor_scalar(eqg[:], colmax2[:], gmax[:], None, OP.is_equal)
            nf = otsu_pool.tile([127, 2], F32, tag="nf")
            nc.vector.scalar_tensor_tensor(nf[:], eqg[:], BIG, nfb[:], OP.mult, OP.add)
            nfm = otsu_pool.tile([127, 1], F32, tag="nfm")
            nc.vector.tensor_reduce(nfm[:], nf[:], AX.X, OP.max)
            nc.gpsimd.partition_all_reduce(nfm[:], nfm[:], channels=127,
                                           reduce_op=bass_isa.ReduceOp.max)

            fl1 = otsu_pool.tile([1, 1], F32, tag="fl1")
            nc.vector.tensor_scalar(fl1[:], nfm[0:1, 0:1], -1.0, None, OP.mult)
            # t1 = floor((flat+0.5)*R254) (margin 0.5/254 >> rounding error)
            qt = otsu_pool.tile([1, 1], F32, tag="qt")
            nc.vector.tensor_scalar(qt[:], fl1[:], 0.5, R254, OP.add, OP.mult)
            t1i = otsu_pool.tile([1, 1], F32, tag="t1i")
            tf1 = otsu_pool.tile([1, 1], F32, tag="tf1")
            _floor(nc, nc.vector, t1i[:], qt[:], tf1[:])
            t2i = otsu_pool.tile([1, 1], F32, tag="t2i")
            nc.vector.scalar_tensor_tensor(t2i[:], t1i[:], -254.0, fl1[:], OP.mult, OP.add)
            # exact thresholds from the table
            selv = otsu_pool.tile([1, NT], F32, tag="selv")
            T1 = otsu_pool.tile([1, 1], F32, tag="T1")
            nc.vector.tensor_scalar(selv[:], iot[:], t1i[:], None, OP.is_equal)
            nc.vector.tensor_tensor(selv[:], selv[:], Ttab[:], OP.mult)
            nc.vector.tensor_reduce(T1[:], selv[:], AX.X, OP.add)
            T2 = otsu_pool.tile([1, 1], F32, tag="T2")
            nc.vector.tensor_scalar(selv[:], iot[:], t2i[:], None, OP.is_equal)
            nc.vector.tensor_tensor(selv[:], selv[:], Ttab[:], OP.mult)
            nc.vector.tensor_reduce(T2[:], selv[:], AX.X, OP.add)
            T1c = otsu_pool.tile([128, 1], F32, tag="T1c")
            nc.gpsimd.partition_broadcast(T1c[:], T1[:], channels=128)
            T2c = otsu_pool.tile([128, 1], F32, tag="T2c")
            nc.gpsimd.partition_broadcast(T2c[:], T2[:], channels=128)

            nc.vector.tensor_copy(out=dbg_row[:, 4 * b:4 * b + 1], in_=fl1[:])
            nc.vector.tensor_copy(out=dbg_row[:, 4 * b + 1:4 * b + 2], in_=ntot[:])
            nc.vector.tensor_copy(out=dbg_row[:, 4 * b + 2:4 * b + 3], in_=T1[:])
            nc.vector.tensor_copy(out=dbg_row[:, 4 * b + 3:4 * b + 4], in_=T2[:])

            # ---------------- MSE ----------------
            if STAGE < 5:
                continue
            for s in range(NSLAB):
                sl = slice(512 * s, 512 * (s + 1))
                prd = prd_pool.tile([128, W], F32, tag="prd")
                nc.sync.dma_start(
                    out=prd[:],
                    in_=prd_d[512 * b + 128 * s:512 * b + 128 * (s + 1), :])
                ge1 = scr_pool.tile([128, W], F32, tag="t0")
                nc.gpsimd.tensor_scalar(ge1[:], img[:, sl], T1c[:], None, OP.is_ge)
                ge2 = scr_pool.tile([128, W], F32, tag="t1")
                nc.gpsimd.tensor_scalar(ge2[:], img[:, sl], T2c[:], None, OP.is_ge)
                nc.vector.scalar_tensor_tensor(ge2[:], ge1[:], 0.5, ge2[:], OP.mult, OP.max)
                pm = scr_pool.tile([128, W], F32, tag="t2")
                nc.vector.tensor_tensor(pm[:], prd[:], M[:, sl], OP.mult)
                nc.vector.tensor_tensor(pm[:], ge2[:], pm[:], OP.subtract)
                dsq = scr_pool.tile([128, W], F32, tag="t3")
                nc.vector.scalar_tensor_tensor(
                    dsq[:], pm[:], 1.0, pm[:], OP.mult, OP.mult,
                    accum_out=sq_cols[:, 4 * b + s:4 * b + s + 1])

        # ---------------- ship stats ----------------
        allc = stat_pool.tile([128, 32], F32, tag="allc")
        nc.vector.tensor_copy(out=allc[:, 0:16], in_=sq_cols[:])
        nc.vector.tensor_copy(out=allc[:, 16:32], in_=sm_cols[:])
        red = stat_pool.tile([128, 32], F32, tag="red")
        nc.gpsimd.partition_all_reduce(red[:], allc[:], channels=128,
                                       reduce_op=bass_isa.ReduceOp.add)
        nc.sync.dma_start(out=out_d[:], in_=red[0:1, :])
        nc.sync.dma_start(out=dbg_d[:], in_=dbg_row[:])


_NC_CACHE = None


def _get_nc():
    global _NC_CACHE
    if _NC_CACHE is None:
        _NC_CACHE = build_nc()
    return _NC_CACHE


def kernel(preds, labels, images):
    preds = np.asarray(preds)
    labels = np.asarray(labels)
    images = np.asarray(images)
    B = preds.shape[0]
    assert B == 32 and preds.shape == (32, 1, 512, 512)
    nc = _get_nc()

    in_maps = []
    for c in range(8):
        sl = slice(B_PER_CORE * c, B_PER_CORE * (c + 1))
        in_maps.append({
            "labels": labels[sl, 0].reshape(B_PER_CORE * H, W),
            "images": images[sl, 0].reshape(B_PER_CORE * H, W),
            "preds": preds[sl, 0].reshape(B_PER_CORE * H, W),
        })
    res = run_bass_kernel_spmd(nc, in_maps, list(range(8)))

    sq = np.zeros(32, np.float32)
    sm = np.zeros(32, np.float32)
    for c in range(8):
        st = res.results[c]["stats"][0]
        for b in range(B_PER_CORE):
            sq[B_PER_CORE * c + b] = np.sum(st[4 * b:4 * b + 4], dtype=np.float32)
            sm[B_PER_CORE * c + b] = np.sum(st[16 + 4 * b:16 + 4 * b + 4], dtype=np.float32)
    smp = (sm + np.float32(EPS)).astype(np.float32)
    valid = smp > np.float32(1e-8)
    loss_per = (sq / smp).astype(np.float32)
    cnt = np.float32(valid.sum())
    if cnt > 0:
        total = np.sum(np.where(valid, loss_per, np.float32(0.0)), dtype=np.float32)
        out = np.float32(total / np.maximum(cnt, np.float32(1.0)))
    else:
        out = np.float32(0.0)
    return np.float32(out)

